# revision 21
# baseline (speedup 1.0000x reference)
"""Trainium2 Bass kernel for additive-attention nn.Module.

Math: reference computes
    scores[b,i,j] = x[b,i,:]@W[0,:3] + key[b,j,:]@W[0,3:] + b0
    attn = softmax(scores, axis=j) ; out = attn @ value

softmax over j is shift-invariant, so the x- and bias-terms (constant in j)
cancel exactly: attn[b,i,j] = softmax_j(key[b,j,:]@W[0,3:]) independent of i.
Hence out[b,i,:] = sum_j p[b,j] * value[b,j,:]  (identical for every i).

Device kernel (data-parallel over batch, 8 batches/core on 8 cores) computes
only the UNNORMALIZED (BPC, DV) row sums per batch plus the softmax
denominators; the host divides and broadcasts along i during unshard.

Layout trick: j is bound to (s, q, jj) with q = SBUF partition, so BOTH the
key and the value arrive already "transposed" (host pre-swizzles; the key is
only 49 KB). The whole e-chain then runs in the j-on-partitions layout:
  sk[q, (f,k,s,jj)] -> 3 fp16 DVE ops on (128,64) -> exp -> eTf (128,64) f32
No PE transposes at all. eTz (128, c, 4) bf16 zero-padded weight blocks are
built with 8 small strided copies (+ one memset); block column m = pos%4
lets every reduction matmul write all 4 PSUM rows of its group (M=4).

Reduction is split across engines by arrival order:
  PE   k0,k2,k4,k6,k7: 8 accumulating matmuls each (fp32 accumulation)
  DVE  k1,k3: bf16 scale + 2 tree adds, folded by 2 all-ones matmuls
  ACT  k5: scale via scalar.mul, adds on DVE, folded on PE
Two PSUM groups of 4 batches; group 0's copy-out + DMA overlap group 1.
W (6 floats) is baked into the compiled kernel as immediates; value is
host-cast to bf16, key to fp16 (errors ~0.4%/0.1% vs the 2e-2 budget).
Per-core device traffic: 4.2 MB value + 49 KB key in, 8.3 KB out.
"""

import numpy as np
from contextlib import ExitStack

import ml_dtypes
import concourse.bass as bass
import concourse.bacc as bacc
import concourse.mybir as mybir
from concourse import tile
from concourse.bass_utils import run_bass_kernel_spmd

B, S1, S2, DV = 64, 1024, 1024, 256
NCORES = 8
BPC = B // NCORES            # batches per core
NS = 2                       # j-halves per batch
JH = S2 // NS                # j per half
NJ = JH // 128               # jj slices per half (4)
NC_ = BPC * NS * NJ          # 64 weight columns (k, s, jj)
GRP = 4                      # batches per PSUM output group
F32 = mybir.dt.float32
F16 = mybir.dt.float16
BF16 = mybir.dt.bfloat16

# batch consumption order (k -> original batch index): the two value DGE
# rings alternate, so completions arrive ring0,ring1,ring0,...
BATCH_ORDER = [0, 4, 1, 5, 2, 6, 3, 7]
DVE_KS = (1, 3)              # batches reduced on DVE
ACT_KS = (5,)                # batches scaled on the scalar engine

_compiled = {}


def _build_nc(wk):
    nc = bacc.Bacc("TRN2", target_bir_lowering=False, debug=False,
                   num_devices=NCORES)

    key_d = nc.dram_tensor("key", [128, 3 * NC_], F16, kind="ExternalInput")
    val_d = nc.dram_tensor("value", [128, BPC, NS * NJ * DV], BF16,
                           kind="ExternalInput")
    out_d = nc.dram_tensor("out", [BPC, DV], F32, kind="ExternalOutput")
    sp_d = nc.dram_tensor("sp", [NC_, 1], F32, kind="ExternalOutput")

    with tile.TileContext(nc) as tc, ExitStack() as ctx:
        const = ctx.enter_context(tc.tile_pool(name="const", bufs=1))
        sm = ctx.enter_context(tc.tile_pool(name="sm", bufs=1))
        ps_s = ctx.enter_context(
            tc.tile_pool(name="ps_s", bufs=1, space=bass.MemorySpace.PSUM))
        ps_o = ctx.enter_context(
            tc.tile_pool(name="ps_o", bufs=2, space=bass.MemorySpace.PSUM))

        # key first on the sync ring (critical path: key -> e -> eTz)
        kq = sm.tile([128, 3 * NC_], F16)
        nc.sync.dma_start(kq[:], key_d[:])
        kq3 = kq[:].rearrange("q (f c) -> q f c", f=3)

        # value: 9 per-batch DMAs (k7 split in half) alternating the sync
        # and gpsimd rings; host already swizzled to (128, k, s*jj*d)
        v_sb = sm.tile([128, BPC, NS * NJ * DV], BF16)
        HV = NS * NJ * DV // 2
        for i, (k, eng, lo, hi) in enumerate((
                (0, nc.sync, 0, 2 * HV), (1, nc.gpsimd, 0, 2 * HV),
                (2, nc.sync, 0, 2 * HV), (3, nc.gpsimd, 0, 2 * HV),
                (4, nc.sync, 0, 2 * HV), (5, nc.gpsimd, 0, 2 * HV),
                (6, nc.sync, 0, 2 * HV),
                (7, nc.gpsimd, 0, HV), (7, nc.gpsimd, HV, 2 * HV))):
            eng.dma_start(v_sb[:, k, lo:hi], val_d.ap()[:, k, lo:hi])

        # constants via memset (no DMA): oc m-blocks for fold matmuls,
        # f32 ones column for the denominator matmul, eTz zero background
        oc = const.tile([128, GRP * GRP], BF16)
        onesf = const.tile([128, 1], F32)
        eTz = sm.tile([128, NC_, GRP], BF16)
        nc.gpsimd.memset(oc[:], 0.0)
        for m in range(GRP):
            nc.gpsimd.memset(oc[:, GRP * m + m:GRP * m + m + 1], 1.0)
        nc.gpsimd.memset(onesf[:], 1.0)
        nc.gpsimd.memset(eTz[:], 0.0)

        # sk = key . w_k in the j-on-partitions layout: (128, 64) fp16 ops
        sk0 = sm.tile([128, NC_], F16)
        sk1 = sm.tile([128, NC_], F16)
        sk2 = sm.tile([128, NC_], F16)
        nc.vector.tensor_scalar_mul(sk0[:], kq3[:, 0, :], float(wk[0]))
        nc.vector.scalar_tensor_tensor(
            sk1[:], kq3[:, 1, :], float(wk[1]), sk0[:],
            op0=mybir.AluOpType.mult, op1=mybir.AluOpType.add)
        nc.vector.scalar_tensor_tensor(
            sk2[:], kq3[:, 2, :], float(wk[2]), sk1[:],
            op0=mybir.AluOpType.mult, op1=mybir.AluOpType.add)

        # e in f32 (feeds everything), zero-padded bf16 blocks via 8 small
        # strided copies: eTz[q, c, m] = e[q, c] for c in k-blocks {m, m+4}
        eTf = sm.tile([128, NC_], F32)
        nc.scalar.activation(eTf[:], sk2[:], mybir.ActivationFunctionType.Exp,
                             bias=0.0, scale=1.0)
        NKC = NS * NJ                      # weight columns per batch
        eTz3 = eTz[:].rearrange("q c m -> q c m")
        with tc.high_priority():
            for m in range(GRP):
                for kb in (m, m + GRP):
                    nc.vector.tensor_copy(
                        eTz3[:, kb * NKC:(kb + 1) * NKC, m],
                        eTf[:, kb * NKC:(kb + 1) * NKC])

        # softmax denominators: per-column partition sums on PE, host adds
        # the 8 columns of each batch
        s_ps = ps_s.tile([NC_, 1], F32)
        nc.tensor.matmul(s_ps[:], eTf[:], onesf[:], start=True, stop=True)
        sp_sb = sm.tile([NC_, 1], F32)
        nc.scalar.activation(sp_sb[:], s_ps[:],
                             mybir.ActivationFunctionType.Copy)
        nc.scalar.dma_start(sp_d.ap(), sp_sb[:])

        # DVE/ACT scale paths for their batches
        v5 = v_sb[:].rearrange("q k (s jj d) -> q k s jj d", s=NS, jj=NJ)
        sc_tiles = {}
        for k in DVE_KS + ACT_KS:
            sc = sm.tile([128, NS * NJ, DV], BF16, tag="sc")
            sc_tiles[k] = sc
            for s in range(NS):
                for jj in range(NJ):
                    c = k * NKC + s * NJ + jj
                    if k in ACT_KS:
                        nc.scalar.mul(sc[:, s * NJ + jj, :],
                                      v5[:, k, s, jj, :], eTf[:, c:c + 1])
                    else:
                        nc.vector.tensor_scalar_mul(
                            sc[:, s * NJ + jj, :], v5[:, k, s, jj, :],
                            eTf[:, c:c + 1])
            nc.vector.tensor_add(sc[:, 0:NJ, :], sc[:, 0:NJ, :],
                                 sc[:, NJ:2 * NJ, :])
            nc.vector.tensor_add(sc[:, 0:2, :], sc[:, 0:2, :], sc[:, 2:4, :])

        # weighted j-reduction on PE: 8 accumulating matmuls per PE batch
        # (lhsT = (128,4) zero-padded e-block, rhs = (128,256) value tile),
        # folds for DVE/ACT batches at the end of each group
        for g in range(BPC // GRP):
            o_ps = ps_o.tile([GRP, DV], F32, tag="o_ps")
            ks = list(range(g * GRP, (g + 1) * GRP))
            mms = []
            for k in ks:
                if k in DVE_KS or k in ACT_KS:
                    continue
                for s in range(NS):
                    for jj in range(NJ):
                        c = k * NKC + s * NJ + jj
                        mms.append((eTz[:, c, :], v5[:, k, s, jj, :]))
            for k in ks:
                if k not in DVE_KS and k not in ACT_KS:
                    continue
                m = k % GRP
                for t in range(2):
                    mms.append((oc[:, GRP * m:GRP * (m + 1)],
                                sc_tiles[k][:, t, :]))
            for i, (lhsT, rhs) in enumerate(mms):
                nc.tensor.matmul(o_ps[:], lhsT, rhs,
                                 start=(i == 0), stop=(i == len(mms) - 1))
            o_sb = sm.tile([GRP, DV], F32, tag="o_sb")
            nc.scalar.activation(o_sb[:], o_ps[:],
                                 mybir.ActivationFunctionType.Copy)
            nc.scalar.dma_start(out_d.ap()[g * GRP:(g + 1) * GRP], o_sb[:])

    nc.compile()
    return nc


def _get_nc(wk):
    key = tuple(float(x) for x in wk)
    if key not in _compiled:
        _compiled[key] = _build_nc(wk)
    return _compiled[key]


def _make_in_maps(key, value):
    key = np.asarray(key, dtype=np.float32)
    value = np.asarray(value, dtype=np.float32).astype(ml_dtypes.bfloat16)

    # key: (B, S2, 3) -> per core (128, f, k, s, jj) fp16 (j-on-partitions)
    k5 = key.reshape(B, NS, 128, NJ, 3).astype(np.float16)

    # value: (B, S2, DV) -> per core (128, k, s*jj*d) in consumption order
    vsw = value.reshape(B, NS, 128, NJ, DV)

    in_maps = []
    for c in range(NCORES):
        lo = c * BPC
        kc = k5[lo:lo + BPC][BATCH_ORDER]           # (k, s, q, jj, f)
        kc = kc.transpose(2, 4, 0, 1, 3).reshape(128, 3 * NC_)
        vc = vsw[lo:lo + BPC][BATCH_ORDER]          # (k, s, q, jj, d)
        vc = vc.transpose(2, 0, 1, 3, 4).reshape(128, BPC, NS * NJ * DV)
        in_maps.append({
            "key": np.ascontiguousarray(kc),
            "value": np.ascontiguousarray(vc),
        })
    return in_maps


def _assemble(results):
    full = np.empty((B, S1, DV), dtype=np.float32)
    for c in range(NCORES):
        raw = results[c]["out"].astype(np.float32)          # (k, DV)
        sp = results[c]["sp"].astype(np.float32).reshape(BPC, NS * NJ)
        s = sp.sum(axis=1)                                  # per k
        for k, b in enumerate(BATCH_ORDER):
            full[c * BPC + b] = (raw[k] / s[k])[None, :]
    return full


def kernel(x, key, value, W, b):
    nc = _get_nc(np.asarray(W, dtype=np.float32)[0, 3:])
    in_maps = _make_in_maps(key, value)
    res = run_bass_kernel_spmd(nc, in_maps, core_ids=list(range(NCORES)))
    return _assemble(res.results)


def kernel_traced(x, key, value, W, b, **spmd_kwargs):
    """Like kernel() but returns (output, BassKernelResults) — for test.py."""
    nc = _get_nc(np.asarray(W, dtype=np.float32)[0, 3:])
    in_maps = _make_in_maps(key, value)
    res = run_bass_kernel_spmd(nc, in_maps, core_ids=list(range(NCORES)),
                               **spmd_kwargs)
    return _assemble(res.results), res


# revision 25
# speedup vs baseline: 1.2513x; 1.2513x over previous
"""Trainium2 Bass kernel for additive-attention nn.Module.

Math: reference computes
    scores[b,i,j] = x[b,i,:]@W[0,:3] + key[b,j,:]@W[0,3:] + b0
    attn = softmax(scores, axis=j) ; out = attn @ value

softmax over j is shift-invariant, so the x- and bias-terms (constant in j)
cancel exactly: attn[b,i,j] = softmax_j(key[b,j,:]@W[0,3:]) independent of i.
Hence out[b,i,:] = sum_j p[b,j] * value[b,j,:]  (identical for every i).

Device kernel (data-parallel over batch, 8 batches/core on 8 cores) computes
only the UNNORMALIZED (BPC, DV) row sums per batch plus the softmax
denominators; the host divides and broadcasts along i during unshard.

Layout trick: j is bound to (s, q, jj) with q = SBUF partition, so BOTH the
key and the value arrive already "transposed" (host pre-swizzles; key is
only 49 KB). The e-chain runs entirely in the j-on-partitions layout:
  sk[q, c] -> 3 fp16 DVE ops on (128,64) -> exp -> eTf (128,64) f32
with weight columns c ordered m-major (m = k%2), so the zero-padded bf16
lhsT blocks eTz[q, c, m] are built with ONE strided copy per m + a memset.

Reduction: per batch 8 accumulating matmuls (lhsT = (128,2) e-block, rhs =
(128,256) value tile, fp32 accumulation) in FOUR PSUM groups of 2 batches
(M=2), so finished groups stream out while later ones accumulate. Batches
k1,k3 are instead scaled+tree-added on DVE (bf16) and folded by 2 all-ones
matmuls emitted late in the PE stream. W is baked into the compiled kernel
as immediates; value is host-cast to bf16, key to fp16 (errors ~0.4%/0.1%
vs the 2e-2 budget). Per-core traffic: 4.2 MB value + 49 KB key in, 8 KB out.
"""

import numpy as np
from contextlib import ExitStack

import ml_dtypes
import concourse.bass as bass
import concourse.bacc as bacc
import concourse.mybir as mybir
from concourse import tile
from concourse.bass_utils import run_bass_kernel_spmd

B, S1, S2, DV = 64, 1024, 1024, 256
NCORES = 8
BPC = B // NCORES            # batches per core
NS = 2                       # j-halves per batch
JH = S2 // NS                # j per half
NJ = JH // 128               # jj slices per half (4)
NKC = NS * NJ                # weight columns per batch (8)
NC_ = BPC * NKC              # 64 weight columns total
GRP = 2                      # batches per PSUM output group (M=2)
NG = BPC // GRP              # 4 groups
F32 = mybir.dt.float32
F16 = mybir.dt.float16
BF16 = mybir.dt.bfloat16

# batch consumption order (k -> original batch index): the two value DGE
# rings alternate, so completions arrive ring0,ring1,ring0,...
BATCH_ORDER = [0, 4, 1, 5, 2, 6, 3, 7]
DVE_KS = (1, 3)              # batches reduced on DVE instead of PE


def _col(k, s, jj):
    """weight-column index: m-major (m = k%GRP), then k//GRP, then (s,jj)."""
    return (k % GRP) * (NG * NKC) + (k // GRP) * NKC + s * NJ + jj


_compiled = {}


def _build_nc(wk):
    nc = bacc.Bacc("TRN2", target_bir_lowering=False, debug=False,
                   num_devices=NCORES)

    key_d = nc.dram_tensor("key", [128, 3 * NC_], F16, kind="ExternalInput")
    val_d = nc.dram_tensor("value", [128, BPC, NS * NJ * DV], BF16,
                           kind="ExternalInput")
    out_d = nc.dram_tensor("out", [BPC, DV], F32, kind="ExternalOutput")
    sp_d = nc.dram_tensor("sp", [NC_, 1], F32, kind="ExternalOutput")

    with tile.TileContext(nc) as tc, ExitStack() as ctx:
        const = ctx.enter_context(tc.tile_pool(name="const", bufs=1))
        sm = ctx.enter_context(tc.tile_pool(name="sm", bufs=1))
        ps_s = ctx.enter_context(
            tc.tile_pool(name="ps_s", bufs=1, space=bass.MemorySpace.PSUM))
        ps_o = ctx.enter_context(
            tc.tile_pool(name="ps_o", bufs=1, space=bass.MemorySpace.PSUM))

        # key first on the sync ring (critical path: key -> e -> eTz)
        kq = sm.tile([128, 3 * NC_], F16)
        nc.sync.dma_start(kq[:], key_d[:])
        kq3 = kq[:].rearrange("q (f c) -> q f c", f=3)

        # constants + eTz background via memset BEFORE gpsimd's DMA issues
        oc = const.tile([128, GRP * GRP], BF16)
        onesf = const.tile([128, 1], F32)
        eTz = sm.tile([128, NC_, GRP], BF16)
        nc.gpsimd.memset(oc[:], 0.0)
        for m in range(GRP):
            nc.gpsimd.memset(oc[:, GRP * m + m:GRP * m + m + 1], 1.0)
        nc.gpsimd.memset(onesf[:], 1.0)
        nc.gpsimd.memset(eTz[:], 0.0)

        # value: 9 DMAs (one per batch, k7 split in half) alternating the
        # sync and gpsimd rings; host already swizzled to (128, k, s*jj*d)
        v_sb = sm.tile([128, BPC, NS * NJ * DV], BF16)
        HV = NS * NJ * DV // 2
        for k, eng, lo, hi in (
                (0, nc.sync, 0, 2 * HV), (1, nc.gpsimd, 0, 2 * HV),
                (2, nc.sync, 0, 2 * HV), (3, nc.gpsimd, 0, 2 * HV),
                (4, nc.sync, 0, 2 * HV), (5, nc.gpsimd, 0, 2 * HV),
                (6, nc.sync, 0, 2 * HV),
                (7, nc.gpsimd, 0, HV), (7, nc.gpsimd, HV, 2 * HV)):
            eng.dma_start(v_sb[:, k, lo:hi], val_d.ap()[:, k, lo:hi])

        # sk = key . w_k in the j-on-partitions layout: (128, 64) fp16 ops
        sk0 = sm.tile([128, NC_], F16)
        sk1 = sm.tile([128, NC_], F16)
        sk2 = sm.tile([128, NC_], F16)
        nc.vector.tensor_scalar_mul(sk0[:], kq3[:, 0, :], float(wk[0]))
        nc.vector.scalar_tensor_tensor(
            sk1[:], kq3[:, 1, :], float(wk[1]), sk0[:],
            op0=mybir.AluOpType.mult, op1=mybir.AluOpType.add)
        nc.vector.scalar_tensor_tensor(
            sk2[:], kq3[:, 2, :], float(wk[2]), sk1[:],
            op0=mybir.AluOpType.mult, op1=mybir.AluOpType.add)

        # e in f32 (feeds everything); zero-padded bf16 blocks via one
        # strided copy per m (columns are m-contiguous by construction)
        eTf = sm.tile([128, NC_], F32)
        nc.scalar.activation(eTf[:], sk2[:], mybir.ActivationFunctionType.Exp,
                             bias=0.0, scale=1.0)
        eTz3 = eTz[:].rearrange("q c m -> q c m")
        MW = NG * NKC                          # columns per m-group (32)
        with tc.high_priority():
            for m in range(GRP):
                nc.vector.tensor_copy(eTz3[:, m * MW:(m + 1) * MW, m],
                                      eTf[:, m * MW:(m + 1) * MW])

        # softmax denominators: per-column partition sums on PE, host adds
        s_ps = ps_s.tile([NC_, 1], F32)
        nc.tensor.matmul(s_ps[:], eTf[:], onesf[:], start=True, stop=True)
        sp_sb = sm.tile([NC_, 1], F32)
        nc.scalar.activation(sp_sb[:], s_ps[:],
                             mybir.ActivationFunctionType.Copy)
        nc.scalar.dma_start(sp_d.ap(), sp_sb[:])

        # DVE scale+tree paths for k1, k3
        v5 = v_sb[:].rearrange("q k (s jj d) -> q k s jj d", s=NS, jj=NJ)
        sc_tiles = {}
        for k in DVE_KS:
            sc = sm.tile([128, NS * NJ, DV], BF16, tag=f"sc{k}")
            sc_tiles[k] = sc
            for s in range(NS):
                for jj in range(NJ):
                    c = _col(k, s, jj)
                    nc.vector.tensor_scalar_mul(
                        sc[:, s * NJ + jj, :], v5[:, k, s, jj, :],
                        eTf[:, c:c + 1])
            nc.vector.tensor_add(sc[:, 0:NJ, :], sc[:, 0:NJ, :],
                                 sc[:, NJ:2 * NJ, :])
            nc.vector.tensor_add(sc[:, 0:2, :], sc[:, 0:2, :], sc[:, 2:4, :])

        # weighted j-reduction on PE. PE stream order: PE batches as their
        # values arrive, DVE folds spliced in late so they never stall the
        # in-order PE queue. 4 PSUM groups (bank per group) interleave.
        o_ps = {g: ps_o.tile([GRP, DV], F32, tag=f"ops{g}", name=f"ops{g}")
                for g in range(NG)}
        started = set()
        mm_count = {g: 0 for g in range(NG)}
        MMS_PER_GROUP = GRP * NKC

        def emit(g, lhsT, rhs):
            nc.tensor.matmul(o_ps[g][:], lhsT, rhs,
                             start=(g not in started),
                             stop=(mm_count[g] == MMS_PER_GROUP - 1),
                             skip_group_check=True)
            started.add(g)
            mm_count[g] += 1

        def emit_pe_batch(k):
            for s in range(NS):
                for jj in range(NJ):
                    c = _col(k, s, jj)
                    emit(k // GRP, eTz3[:, c, :], v5[:, k, s, jj, :])

        def emit_fold(k):
            m = k % GRP
            for t in range(2):
                emit(k // GRP, oc[:, GRP * m:GRP * (m + 1)],
                     sc_tiles[k][:, t, :])

        done_groups = []

        def finish_group(g):
            o_sb = sm.tile([GRP, DV], F32, tag=f"osb{g}")
            nc.scalar.activation(o_sb[:], o_ps[g][:],
                                 mybir.ActivationFunctionType.Copy)
            nc.scalar.dma_start(out_d.ap()[g * GRP:(g + 1) * GRP], o_sb[:])
            done_groups.append(g)

        emit_pe_batch(0)
        emit_pe_batch(2)
        emit_pe_batch(4)
        emit_fold(1)          # g0 complete
        finish_group(0)
        emit_pe_batch(5)      # g2 complete
        finish_group(2)
        emit_fold(3)          # g1 complete
        finish_group(1)
        emit_pe_batch(6)
        emit_pe_batch(7)      # g3 complete
        finish_group(3)

    nc.compile()
    return nc


def _get_nc(wk):
    key = tuple(float(x) for x in wk)
    if key not in _compiled:
        _compiled[key] = _build_nc(wk)
    return _compiled[key]


def _make_in_maps(key, value):
    key = np.asarray(key, dtype=np.float32)
    value = np.asarray(value, dtype=np.float32).astype(ml_dtypes.bfloat16)

    # key: (B, S2, 3) -> per core (128, f, c) fp16, c = m-major column order
    k5 = key.reshape(B, NS, 128, NJ, 3).astype(np.float16)

    # value: (B, S2, DV) -> per core (128, k, s*jj*d) in consumption order
    vsw = value.reshape(B, NS, 128, NJ, DV)

    # column permutation: c-th column is batch korder[c//8], s, jj
    col_k = [0] * NC_
    col_s = [0] * NC_
    col_jj = [0] * NC_
    for k in range(BPC):
        for s in range(NS):
            for jj in range(NJ):
                c = _col(k, s, jj)
                col_k[c], col_s[c], col_jj[c] = k, s, jj

    in_maps = []
    for c0 in range(NCORES):
        lo = c0 * BPC
        kc = k5[lo:lo + BPC][BATCH_ORDER]           # (k, s, q, jj, f)
        # build (128, 3, 64) by gathering columns in c-order
        kcols = np.empty((128, 3, NC_), dtype=np.float16)
        for c in range(NC_):
            kcols[:, :, c] = kc[col_k[c], col_s[c], :, col_jj[c], :]
        vc = vsw[lo:lo + BPC][BATCH_ORDER]          # (k, s, q, jj, d)
        vc = vc.transpose(2, 0, 1, 3, 4).reshape(128, BPC, NS * NJ * DV)
        in_maps.append({
            "key": np.ascontiguousarray(kcols.reshape(128, 3 * NC_)),
            "value": np.ascontiguousarray(vc),
        })
    return in_maps


def _assemble(results):
    full = np.empty((B, S1, DV), dtype=np.float32)
    for c0 in range(NCORES):
        raw = results[c0]["out"].astype(np.float32)          # (k, DV)
        sp = results[c0]["sp"].astype(np.float32).reshape(NC_)
        for k, b in enumerate(BATCH_ORDER):
            s = sum(sp[_col(k, si, jj)] for si in range(NS)
                    for jj in range(NJ))
            full[c0 * BPC + b] = (raw[k] / s)[None, :]
    return full


def kernel(x, key, value, W, b):
    nc = _get_nc(np.asarray(W, dtype=np.float32)[0, 3:])
    in_maps = _make_in_maps(key, value)
    res = run_bass_kernel_spmd(nc, in_maps, core_ids=list(range(NCORES)))
    return _assemble(res.results)


def kernel_traced(x, key, value, W, b, **spmd_kwargs):
    """Like kernel() but returns (output, BassKernelResults) — for test.py."""
    nc = _get_nc(np.asarray(W, dtype=np.float32)[0, 3:])
    in_maps = _make_in_maps(key, value)
    res = run_bass_kernel_spmd(nc, in_maps, core_ids=list(range(NCORES)),
                               **spmd_kwargs)
    return _assemble(res.results), res


# revision 26
# speedup vs baseline: 1.2540x; 1.0022x over previous
"""Trainium2 Bass kernel for additive-attention nn.Module.

Math: reference computes
    scores[b,i,j] = x[b,i,:]@W[0,:3] + key[b,j,:]@W[0,3:] + b0
    attn = softmax(scores, axis=j) ; out = attn @ value

softmax over j is shift-invariant, so the x- and bias-terms (constant in j)
cancel exactly: attn[b,i,j] = softmax_j(key[b,j,:]@W[0,3:]) independent of i.
Hence out[b,i,:] = sum_j p[b,j] * value[b,j,:]  (identical for every i).

Device kernel (data-parallel over batch, 8 batches/core on 8 cores) computes
only the UNNORMALIZED (BPC, DV) row sums per batch plus the softmax
denominators; the host divides and broadcasts along i during unshard.

Layout trick: j is bound to (s, q, jj) with q = SBUF partition, so BOTH the
key and the value arrive already "transposed" (host pre-swizzles; key is
only 49 KB). The e-chain runs entirely in the j-on-partitions layout:
  sk[q, c] -> 3 fp16 DVE ops on (128,64) -> exp -> eTf (128,64) f32
with weight columns c ordered m-major (m = k%2), so the zero-padded bf16
lhsT blocks eTz[q, c, m] are built with ONE strided copy per m + a memset.

Reduction: per batch 8 accumulating matmuls (lhsT = (128,2) e-block, rhs =
(128,256) value tile, fp32 accumulation) in FOUR PSUM groups of 2 batches
(M=2), so finished groups stream out while later ones accumulate. Batches
k1,k3 are instead scaled+tree-added on DVE (bf16) and folded by 2 all-ones
matmuls emitted late in the PE stream. W is baked into the compiled kernel
as immediates; value is host-cast to bf16, key to fp16 (errors ~0.4%/0.1%
vs the 2e-2 budget). Per-core traffic: 4.2 MB value + 49 KB key in, 8 KB out.
"""

import numpy as np
from contextlib import ExitStack

import ml_dtypes
import concourse.bass as bass
import concourse.bacc as bacc
import concourse.mybir as mybir
from concourse import tile
from concourse.bass_utils import run_bass_kernel_spmd

B, S1, S2, DV = 64, 1024, 1024, 256
NCORES = 8
BPC = B // NCORES            # batches per core
NS = 2                       # j-halves per batch
JH = S2 // NS                # j per half
NJ = JH // 128               # jj slices per half (4)
NKC = NS * NJ                # weight columns per batch (8)
NC_ = BPC * NKC              # 64 weight columns total
GRP = 2                      # batches per PSUM output group (M=2)
NG = BPC // GRP              # 4 groups
F32 = mybir.dt.float32
F16 = mybir.dt.float16
BF16 = mybir.dt.bfloat16

# batch consumption order (k -> original batch index): the two value DGE
# rings alternate, so completions arrive ring0,ring1,ring0,...
BATCH_ORDER = [0, 4, 1, 5, 2, 6, 3, 7]
DVE_KS = (1, 3)              # batches reduced on DVE instead of PE


def _col(k, s, jj):
    """weight-column index: m-major (m = k%GRP), then k//GRP, then (s,jj)."""
    return (k % GRP) * (NG * NKC) + (k // GRP) * NKC + s * NJ + jj


_compiled = {}


def _build_nc(wk):
    nc = bacc.Bacc("TRN2", target_bir_lowering=False, debug=False,
                   num_devices=NCORES)

    key_d = nc.dram_tensor("key", [128, 3 * NC_], F16, kind="ExternalInput")
    val_d = nc.dram_tensor("value", [128, BPC, NS * NJ * DV], BF16,
                           kind="ExternalInput")
    out_d = nc.dram_tensor("out", [BPC, DV], F32, kind="ExternalOutput")
    sp_d = nc.dram_tensor("sp", [NC_, 1], F32, kind="ExternalOutput")

    with tile.TileContext(nc) as tc, ExitStack() as ctx:
        const = ctx.enter_context(tc.tile_pool(name="const", bufs=1))
        sm = ctx.enter_context(tc.tile_pool(name="sm", bufs=1))
        ps_s = ctx.enter_context(
            tc.tile_pool(name="ps_s", bufs=1, space=bass.MemorySpace.PSUM))
        ps_o = ctx.enter_context(
            tc.tile_pool(name="ps_o", bufs=1, space=bass.MemorySpace.PSUM))

        # key first on the sync ring (critical path: key -> e -> eTz)
        kq = sm.tile([128, 3 * NC_], F16)
        nc.sync.dma_start(kq[:], key_d[:])
        kq3 = kq[:].rearrange("q (f c) -> q f c", f=3)

        # value k0 first on the gpsimd ring (it gates the PE start), then
        # the memset constants, then the remaining value DMAs alternating
        # rings in consumption order (k6/k7 split in half for a finer tail)
        v_sb = sm.tile([128, BPC, NS * NJ * DV], BF16)
        HV = NS * NJ * DV // 2
        nc.gpsimd.dma_start(v_sb[:, 0, :], val_d.ap()[:, 0, :])

        oc = const.tile([128, GRP * GRP], BF16)
        onesf = const.tile([128, 1], F32)
        eTz = sm.tile([128, NC_, GRP], BF16)
        nc.gpsimd.memset(oc[:], 0.0)
        for m in range(GRP):
            nc.gpsimd.memset(oc[:, GRP * m + m:GRP * m + m + 1], 1.0)
        nc.gpsimd.memset(onesf[:], 1.0)
        nc.gpsimd.memset(eTz[:], 0.0)

        for k, eng, lo, hi in (
                (1, nc.sync, 0, 2 * HV), (2, nc.gpsimd, 0, 2 * HV),
                (3, nc.sync, 0, 2 * HV), (4, nc.gpsimd, 0, 2 * HV),
                (5, nc.sync, 0, 2 * HV),
                (6, nc.gpsimd, 0, HV), (6, nc.gpsimd, HV, 2 * HV),
                (7, nc.sync, 0, HV), (7, nc.sync, HV, 2 * HV)):
            eng.dma_start(v_sb[:, k, lo:hi], val_d.ap()[:, k, lo:hi])

        # sk = key . w_k in the j-on-partitions layout: (128, 64) fp16 ops
        sk0 = sm.tile([128, NC_], F16)
        sk1 = sm.tile([128, NC_], F16)
        sk2 = sm.tile([128, NC_], F16)
        nc.vector.tensor_scalar_mul(sk0[:], kq3[:, 0, :], float(wk[0]))
        nc.vector.scalar_tensor_tensor(
            sk1[:], kq3[:, 1, :], float(wk[1]), sk0[:],
            op0=mybir.AluOpType.mult, op1=mybir.AluOpType.add)
        nc.vector.scalar_tensor_tensor(
            sk2[:], kq3[:, 2, :], float(wk[2]), sk1[:],
            op0=mybir.AluOpType.mult, op1=mybir.AluOpType.add)

        # e in f32 (feeds everything); zero-padded bf16 blocks via one
        # strided copy per m (columns are m-contiguous by construction)
        eTf = sm.tile([128, NC_], F32)
        nc.scalar.activation(eTf[:], sk2[:], mybir.ActivationFunctionType.Exp,
                             bias=0.0, scale=1.0)
        eTz3 = eTz[:].rearrange("q c m -> q c m")
        MW = NG * NKC                          # columns per m-group (32)
        with tc.high_priority():
            for m in range(GRP):
                nc.vector.tensor_copy(eTz3[:, m * MW:(m + 1) * MW, m],
                                      eTf[:, m * MW:(m + 1) * MW])

        # softmax denominators: per-column partition sums on PE, host adds
        s_ps = ps_s.tile([NC_, 1], F32)
        nc.tensor.matmul(s_ps[:], eTf[:], onesf[:], start=True, stop=True)
        sp_sb = sm.tile([NC_, 1], F32)
        nc.scalar.activation(sp_sb[:], s_ps[:],
                             mybir.ActivationFunctionType.Copy)
        nc.scalar.dma_start(sp_d.ap(), sp_sb[:])

        # DVE scale+tree paths for k1, k3
        v5 = v_sb[:].rearrange("q k (s jj d) -> q k s jj d", s=NS, jj=NJ)
        sc_tiles = {}
        for k in DVE_KS:
            sc = sm.tile([128, NS * NJ, DV], BF16, tag=f"sc{k}")
            sc_tiles[k] = sc
            for s in range(NS):
                for jj in range(NJ):
                    c = _col(k, s, jj)
                    nc.vector.tensor_scalar_mul(
                        sc[:, s * NJ + jj, :], v5[:, k, s, jj, :],
                        eTf[:, c:c + 1])
            nc.vector.tensor_add(sc[:, 0:NJ, :], sc[:, 0:NJ, :],
                                 sc[:, NJ:2 * NJ, :])
            nc.vector.tensor_add(sc[:, 0:2, :], sc[:, 0:2, :], sc[:, 2:4, :])

        # weighted j-reduction on PE. PE stream order: PE batches as their
        # values arrive, DVE folds spliced in late so they never stall the
        # in-order PE queue. 4 PSUM groups (bank per group) interleave.
        o_ps = {g: ps_o.tile([GRP, DV], F32, tag=f"ops{g}", name=f"ops{g}")
                for g in range(NG)}
        started = set()
        mm_count = {g: 0 for g in range(NG)}
        MMS_PER_GROUP = GRP * NKC

        def emit(g, lhsT, rhs):
            nc.tensor.matmul(o_ps[g][:], lhsT, rhs,
                             start=(g not in started),
                             stop=(mm_count[g] == MMS_PER_GROUP - 1),
                             skip_group_check=True)
            started.add(g)
            mm_count[g] += 1

        def emit_pe_batch(k):
            for s in range(NS):
                for jj in range(NJ):
                    c = _col(k, s, jj)
                    emit(k // GRP, eTz3[:, c, :], v5[:, k, s, jj, :])

        def emit_fold(k):
            m = k % GRP
            for t in range(2):
                emit(k // GRP, oc[:, GRP * m:GRP * (m + 1)],
                     sc_tiles[k][:, t, :])

        done_groups = []

        def finish_group(g):
            o_sb = sm.tile([GRP, DV], F32, tag=f"osb{g}")
            nc.scalar.activation(o_sb[:], o_ps[g][:],
                                 mybir.ActivationFunctionType.Copy)
            nc.scalar.dma_start(out_d.ap()[g * GRP:(g + 1) * GRP], o_sb[:])
            done_groups.append(g)

        emit_pe_batch(0)
        emit_pe_batch(2)
        emit_pe_batch(4)
        emit_fold(1)          # g0 complete
        finish_group(0)
        emit_pe_batch(5)      # g2 complete
        finish_group(2)
        emit_fold(3)          # g1 complete
        finish_group(1)
        emit_pe_batch(6)
        emit_pe_batch(7)      # g3 complete
        finish_group(3)

    nc.compile()
    return nc


def _get_nc(wk):
    key = tuple(float(x) for x in wk)
    if key not in _compiled:
        _compiled[key] = _build_nc(wk)
    return _compiled[key]


def _make_in_maps(key, value):
    key = np.asarray(key, dtype=np.float32)
    value = np.asarray(value, dtype=np.float32).astype(ml_dtypes.bfloat16)

    # key: (B, S2, 3) -> per core (128, f, c) fp16, c = m-major column order
    k5 = key.reshape(B, NS, 128, NJ, 3).astype(np.float16)

    # value: (B, S2, DV) -> per core (128, k, s*jj*d) in consumption order
    vsw = value.reshape(B, NS, 128, NJ, DV)

    # column permutation: c-th column is batch korder[c//8], s, jj
    col_k = [0] * NC_
    col_s = [0] * NC_
    col_jj = [0] * NC_
    for k in range(BPC):
        for s in range(NS):
            for jj in range(NJ):
                c = _col(k, s, jj)
                col_k[c], col_s[c], col_jj[c] = k, s, jj

    in_maps = []
    for c0 in range(NCORES):
        lo = c0 * BPC
        kc = k5[lo:lo + BPC][BATCH_ORDER]           # (k, s, q, jj, f)
        # build (128, 3, 64) by gathering columns in c-order
        kcols = np.empty((128, 3, NC_), dtype=np.float16)
        for c in range(NC_):
            kcols[:, :, c] = kc[col_k[c], col_s[c], :, col_jj[c], :]
        vc = vsw[lo:lo + BPC][BATCH_ORDER]          # (k, s, q, jj, d)
        vc = vc.transpose(2, 0, 1, 3, 4).reshape(128, BPC, NS * NJ * DV)
        in_maps.append({
            "key": np.ascontiguousarray(kcols.reshape(128, 3 * NC_)),
            "value": np.ascontiguousarray(vc),
        })
    return in_maps


def _assemble(results):
    full = np.empty((B, S1, DV), dtype=np.float32)
    for c0 in range(NCORES):
        raw = results[c0]["out"].astype(np.float32)          # (k, DV)
        sp = results[c0]["sp"].astype(np.float32).reshape(NC_)
        for k, b in enumerate(BATCH_ORDER):
            s = sum(sp[_col(k, si, jj)] for si in range(NS)
                    for jj in range(NJ))
            full[c0 * BPC + b] = (raw[k] / s)[None, :]
    return full


def kernel(x, key, value, W, b):
    nc = _get_nc(np.asarray(W, dtype=np.float32)[0, 3:])
    in_maps = _make_in_maps(key, value)
    res = run_bass_kernel_spmd(nc, in_maps, core_ids=list(range(NCORES)))
    return _assemble(res.results)


def kernel_traced(x, key, value, W, b, **spmd_kwargs):
    """Like kernel() but returns (output, BassKernelResults) — for test.py."""
    nc = _get_nc(np.asarray(W, dtype=np.float32)[0, 3:])
    in_maps = _make_in_maps(key, value)
    res = run_bass_kernel_spmd(nc, in_maps, core_ids=list(range(NCORES)),
                               **spmd_kwargs)
    return _assemble(res.results), res


# revision 36
# speedup vs baseline: 1.4113x; 1.1255x over previous
"""Round-2 kernel (best measured 34.2us): scatter-matmul e-chain, 5 coarse
value DMAs on 2 rings, PE-only reduction (M=4, 2 PSUM groups)."""

import numpy as np
from contextlib import ExitStack

import ml_dtypes
import concourse.bass as bass
import concourse.bacc as bacc
import concourse.mybir as mybir
from concourse import tile
from concourse.bass_utils import run_bass_kernel_spmd

B, S1, S2, DV = 64, 1024, 1024, 256
NCORES = 8
BPC = B // NCORES
NS = 2
NP = BPC * NS
JH = S2 // NS
NJ = JH // 128
GRP = 4
F32 = mybir.dt.float32
BF16 = mybir.dt.bfloat16

BATCH_ORDER = [0, 4, 1, 5, 2, 6, 3, 7]

_compiled = {}


def _build_nc():
    nc = bacc.Bacc("TRN2", target_bir_lowering=False, debug=False,
                   num_devices=NCORES)

    key_d = nc.dram_tensor("key", [NP, 3 * (JH + 1)], F32,
                           kind="ExternalInput")
    val_d = nc.dram_tensor("value", [128, BPC, NS * NJ * DV], BF16,
                           kind="ExternalInput")
    sct_d = nc.dram_tensor("scat", [NP, NP * GRP], BF16, kind="ExternalInput")
    out_d = nc.dram_tensor("out", [BPC, DV], F32, kind="ExternalOutput")
    sp_d = nc.dram_tensor("sp", [NP, 1], F32, kind="ExternalOutput")

    with tile.TileContext(nc) as tc, ExitStack() as ctx:
        const = ctx.enter_context(tc.tile_pool(name="const", bufs=1))
        sm = ctx.enter_context(tc.tile_pool(name="sm", bufs=1))
        ps_tp = ctx.enter_context(
            tc.tile_pool(name="ps_tp", bufs=2, space=bass.MemorySpace.PSUM))
        ps_o = ctx.enter_context(
            tc.tile_pool(name="ps_o", bufs=2, space=bass.MemorySpace.PSUM))

        k_sb = sm.tile([NP, 3 * (JH + 1)], F32)
        nc.sync.dma_start(k_sb[:], key_d[:])
        k3 = k_sb[:].rearrange("p (f j) -> p f j", f=3)

        scat_sb = const.tile([NP, NP * GRP], BF16)
        nc.scalar.dma_start(scat_sb[:], sct_d[:])

        v_sb = sm.tile([128, BPC, NS * NJ * DV], BF16)
        for ks, eng in (((0,), nc.sync), ((1,), nc.gpsimd),
                        ((2, 3), nc.sync), ((4, 5), nc.gpsimd),
                        ((6, 7), nc.sync)):
            lo, hi = ks[0], ks[-1] + 1
            eng.dma_start(v_sb[:, lo:hi, :], val_d.ap()[:, lo:hi, :])

        sk0 = sm.tile([NP, JH], F32)
        sk1 = sm.tile([NP, JH], F32)
        sk2 = sm.tile([NP, JH], F32)
        nc.vector.tensor_scalar_mul(sk0[:], k3[:, 0, 1:], k3[:, 0, 0:1])
        nc.vector.scalar_tensor_tensor(
            sk1[:], k3[:, 1, 1:], k3[:, 1, 0:1], sk0[:],
            op0=mybir.AluOpType.mult, op1=mybir.AluOpType.add)
        nc.vector.scalar_tensor_tensor(
            sk2[:], k3[:, 2, 1:], k3[:, 2, 0:1], sk1[:],
            op0=mybir.AluOpType.mult, op1=mybir.AluOpType.add)

        e = sm.tile([NP, JH], BF16)
        sp = sm.tile([NP, 1], F32)
        nc.scalar.activation(e[:], sk2[:], mybir.ActivationFunctionType.Exp,
                             bias=0.0, scale=1.0, accum_out=sp[:])

        e_il = e[:].rearrange("p (q jj) -> p jj q", jj=NJ)
        eTz = sm.tile([128, NJ, NP * GRP], BF16)
        for jj in range(NJ):
            tp = ps_tp.tile([128, NP * GRP], F32)
            nc.tensor.matmul(tp[:], e_il[:, jj, :], scat_sb[:],
                             start=True, stop=True)
            nc.vector.tensor_copy(eTz[:, jj, :], tp[:])

        nc.scalar.dma_start(sp_d.ap(), sp[:])

        v5 = v_sb[:].rearrange("q k (s jj d) -> q k s jj d", s=NS, jj=NJ)
        for g in range(BPC // GRP):
            o_ps = ps_o.tile([GRP, DV], F32, tag="o_ps")
            ks = list(range(g * GRP, (g + 1) * GRP))
            nmm = 0
            for k in ks:
                b = BATCH_ORDER[k]
                for s in range(NS):
                    for jj in range(NJ):
                        p = NS * b + s
                        nc.tensor.matmul(
                            o_ps[:], eTz[:, jj, GRP * p:GRP * (p + 1)],
                            v5[:, k, s, jj, :],
                            start=(nmm == 0),
                            stop=(nmm == GRP * NS * NJ - 1))
                        nmm += 1
            o_sb = sm.tile([GRP, DV], F32, tag=f"osb{g}")
            nc.scalar.activation(o_sb[:], o_ps[:],
                                 mybir.ActivationFunctionType.Copy)
            nc.sync.dma_start(out_d.ap()[g * GRP:(g + 1) * GRP], o_sb[:])

    nc.compile()
    return nc


def _get_nc():
    if "nc" not in _compiled:
        _compiled["nc"] = _build_nc()
    return _compiled["nc"]


def _make_in_maps(key, value, W):
    key = np.asarray(key, dtype=np.float32)
    value = np.asarray(value, dtype=np.float32).astype(ml_dtypes.bfloat16)
    W = np.asarray(W, dtype=np.float32)

    kT = key.reshape(B, NS, JH, 3).transpose(0, 1, 3, 2)
    kaug = np.empty((B, NS, 3, JH + 1), dtype=np.float32)
    kaug[..., 0] = W[0, 3:].reshape(1, 1, 3)
    kaug[..., 1:] = kT

    vsw = value.reshape(B, NS, 128, NJ, DV)

    pos = {b: k for k, b in enumerate(BATCH_ORDER)}
    scat = np.zeros((NP, NP * GRP), dtype=np.float32)
    for p in range(NP):
        scat[p, GRP * p + pos[p // NS] % GRP] = 1.0
    scat = scat.astype(ml_dtypes.bfloat16)

    in_maps = []
    for c in range(NCORES):
        lo = c * BPC
        kc = kaug[lo:lo + BPC].reshape(NP, 3 * (JH + 1))
        vc = vsw[lo:lo + BPC][BATCH_ORDER]
        vc = vc.transpose(2, 0, 1, 3, 4).reshape(128, BPC, NS * NJ * DV)
        in_maps.append({
            "key": np.ascontiguousarray(kc),
            "value": np.ascontiguousarray(vc),
            "scat": scat,
        })
    return in_maps


def _assemble(results):
    full = np.empty((B, S1, DV), dtype=np.float32)
    for c in range(NCORES):
        raw = results[c]["out"].astype(np.float32)
        sp = results[c]["sp"].astype(np.float32).reshape(BPC, NS).sum(axis=1)
        for k, b in enumerate(BATCH_ORDER):
            full[c * BPC + b] = (raw[k] / sp[b])[None, :]
    return full


def kernel(x, key, value, W, b):
    nc = _get_nc()
    in_maps = _make_in_maps(key, value, W)
    res = run_bass_kernel_spmd(nc, in_maps, core_ids=list(range(NCORES)))
    return _assemble(res.results)


def kernel_traced(x, key, value, W, b, **spmd_kwargs):
    nc = _get_nc()
    in_maps = _make_in_maps(key, value, W)
    res = run_bass_kernel_spmd(nc, in_maps, core_ids=list(range(NCORES)),
                               **spmd_kwargs)
    return _assemble(res.results), res
